# revision 60
# baseline (speedup 1.0000x reference)
"""Trainium2 Bass kernel for BitLTIInjection (BitNet-style fake-quantized linear
+ LTI injection):

    A_eff = 0.99*tanh(A_raw)
    e_q   = per-token absmax int8 fake quant of e
    W_q   = absmean ternary fake quant of W
    out   = A_eff*h + e_q @ W_q.T + block_out

Data-parallel over B*T across 8 cores; W replicated.

Primary variant ("fp8"): the quantized matmul runs in fp8e4m3 with the
DoubleRow (double-pumped) PE mode, contracting two 128-chunks per pass.
W_q is ternary {-1,0,1} (exact in fp8); e_q loses ~1.5e-2 relative error
from fp8 rounding of int8 values, well inside the 2e-2 gate.  Operands
use the classic chunk layout [128, n_chunks, free] (d = 128*dc + p) with
the pair dim sliced [2c:2c+2] — the hardware's dual-fp8 LDWEIGHTS ISA
check rejects pair-interleaved layouts, so transposes run in bf16 and a
cheap convert pass produces the fp8 copies.

Fallback variant ("bf16"): numerically exact bf16 matmul (all quantized
values are small ints, bf16 + f32 PSUM is exact), same scheduling.

Scheduling: W loads stream on the Pool DMA queue with the absmean
reduce overlapping on DVE; the first three e blocks are loaded and
quantized during that window (only the dequant scalar needs the global
W scale, so it is deferred).  Ternarize is a 3-pass pipeline: pass1
relu(w*s + 1.49) on ACT (the low-clip reshift folds into the magic
constant), pass2 +190.51 bf16 magic-round on DVE, XBAR transpose on SP,
pass3 -192/min(1) -> fp8 on DVE.  wqt is split per output-column block
so ob=0 matmuls unblock after four W tiles.  e loads ride the SP queue,
out-stores the ACT queue, bo/W the Pool queue; no-sync dep hints keep
reloads behind the W load stream and keep each block's tail ops (cvt,
epilogues) from head-of-line-blocking the next block's quant chain.
Most W f32 tiles stay SBUF-resident between absmean and ternarize.
"""

import os

import numpy as np

import concourse.bass as bass
import concourse.mybir as mybir
import concourse.tile as tile
from concourse.bass import ts
from concourse.bass_utils import run_bass_kernel_spmd
from concourse.tile_rust import add_dep_helper

P = 128
MAGIC = 12582912.0   # 1.5 * 2**23: forces RNE-to-integer in f32
MAGIC16 = 192.0      # 1.5 * 2**7: forces RNE-to-integer in bf16 for |x| < 64
EPS = 1e-5
N_CORES = 8
F32 = mybir.dt.float32
BF16 = mybir.dt.bfloat16
FP8 = mybir.dt.float8e4
U16 = mybir.dt.uint16
MM_N = 512  # moving free dim per matmul (one PSUM bank of f32)
CLIP_LO = 1.49  # pre-round low clip offset (relu(w*s + 1.49))
MAGIC2 = float(np.float32(MAGIC16) - np.float32(CLIP_LO))


_SCALE_LAST = [None]


def _scale_chain(nc, pool, m_in, tag, mul):
    """1/m_in via reciprocal + one Newton step, times mul.  Returns AP."""
    r0 = pool.tile([P, 1], F32, tag=f"{tag}_r0")
    nc.vector.reciprocal(r0[:], m_in[:])
    t1 = pool.tile([P, 1], F32, tag=f"{tag}_t1")
    nc.vector.scalar_tensor_tensor(
        out=t1[:], in0=m_in[:], scalar=-1.0, in1=r0[:],
        op0=mybir.AluOpType.mult, op1=mybir.AluOpType.mult,
    )
    nc.vector.tensor_scalar_add(t1[:], t1[:], 2.0)
    op = nc.vector.tensor_scalar_mul(r0[:], r0[:], t1[:])
    if mul != 1.0:
        op = nc.vector.tensor_scalar_mul(r0[:], r0[:], mul)
    _SCALE_LAST[0] = op
    return r0


def build_kernel_body(tc: tile.TileContext, io: dict, Tc: int, D: int,
                      with_h: bool, use_fp8: bool):
    nc = tc.nc
    n_tb = Tc // P   # token blocks per core
    n_wt = D // P    # weight row tiles
    n_ob = D // MM_N  # output column blocks
    n_pc = D // 256  # fp8 pair-chunks
    n_dc = D // P    # bf16 contraction chunks
    HALF = D // 2

    # W f32 tiles resident in SBUF between absmean and ternarize; the rest
    # stream through 2 slots and are re-read from HBM for ternarize.
    N_RES = 9 if use_fp8 else 6

    e_d = io["e"]
    bo_d = io["bo"]
    w_d = io["w"]
    out_d = io["out"]

    with (
        tc.tile_pool(name="wres", bufs=N_RES) as wres_pool,
        tc.tile_pool(name="wstr", bufs=2) as wstr_pool,
        tc.tile_pool(name="wq", bufs=4) as wq_pool,
        tc.tile_pool(name="wz", bufs=3) as wz_pool,
        tc.tile_pool(name="wzT", bufs=3) as wzT_pool,
        tc.tile_pool(name="eT8", bufs=6) as eT8_pool,
        tc.tile_pool(name="scal", bufs=1) as scal_pool,
        tc.tile_pool(name="st", bufs=5) as st_pool,
        tc.tile_pool(name="ef", bufs=3) as ef_pool,
        tc.tile_pool(name="qb", bufs=2) as qb_pool,
        tc.tile_pool(name="eT", bufs=3) as eT_pool,
        tc.tile_pool(name="bo", bufs=2) as bo_pool,
        tc.tile_pool(name="pp", bufs=8, space="PSUM") as pp_pool,
    ):
        # constants
        ones_col = scal_pool.tile([P, 1], F32, tag="ones_col")
        nc.vector.memset(ones_col[:], 1.0)
        ones_row = scal_pool.tile([1, P], F32, tag="ones_row")
        nc.vector.memset(ones_row[:], 1.0)
        clip_lo = scal_pool.tile([P, 1], F32, tag="clip_lo")
        nc.vector.memset(clip_lo[:], CLIP_LO)
        posmagic = scal_pool.tile([P, 1], F32, tag="posmagic")
        nc.vector.memset(posmagic[:], MAGIC)
        negmagic = scal_pool.tile([P, 1], F32, tag="negmagic")
        nc.vector.memset(negmagic[:], -MAGIC)

        # ---------------- W load + absmean (overlapped) ----------------
        # The first N_RES tiles stay SBUF-resident; the last few stream
        # through 2 slots and are re-read from HBM for ternarize.  All
        # reduces on DVE (no cross-engine wait can wedge the pipeline).
        parts = scal_pool.tile([P, n_wt], F32, tag="parts")
        wf_tiles = {}
        # ALL HBM DMA goes through the Pool queue: a single queue sustains
        # ~357 GB/s while splitting traffic across queues DROPS aggregate
        # throughput (measured).  The FIFO order is pinned with no-sync
        # dep hints so a slot-blocked dispatch can't reorder ahead.
        def pool_dma(out_ap, in_ap):
            return nc.gpsimd.dma_start(out=out_ap, in_=in_ap)

        ef_tiles = {}
        wlast = [None]
        for j in range(n_wt):
            pool = wres_pool if j < N_RES else wstr_pool
            wf = pool.tile([P, D], F32, tag="wf32", name=f"wfm_{j}")
            wlast[0] = pool_dma(wf[:], w_d[ts(j, P), :])
            nc.vector.tensor_reduce(
                out=parts[:, j : j + 1], in_=wf[:],
                axis=mybir.AxisListType.X, op=mybir.AluOpType.add,
                apply_absolute_value=True,
            )
            if j < N_RES:
                wf_tiles[j] = wf
            # interleave the first three e loads so the pre-quant chain
            # (blocks 0-2) can run during the W-load window
            if j in (2, 5, 8) and (j - 2) // 3 < n_tb:
                i = (j - 2) // 3
                ef = ef_pool.tile([P, D], F32, tag="ef", name=f"ef_{i}")
                nc.sync.dma_start(out=ef[:], in_=e_d[ts(i, P), :])
                ef_tiles[i] = ef

        bo_tiles = {}
        bo0 = bo_pool.tile([P, D], F32, tag="bo", name="bo_0")
        pool_dma(bo0[:], bo_d[ts(0, P), :])
        bo_tiles[0] = bo0

        # absmean finalize: cross-partition sum + broadcast via tiny PE ops
        acc = scal_pool.tile([P, 1], F32, tag="acc")
        nc.vector.tensor_reduce(
            out=acc[:], in_=parts[:], axis=mybir.AxisListType.X,
            op=mybir.AluOpType.add,
        )
        tot_ps = pp_pool.tile([P, MM_N], F32, tag="ps", name="tot_ps")
        nc.tensor.matmul(tot_ps[:1, :1], ones_col[:], acc[:])
        tot_sb = scal_pool.tile([1, 1], F32, tag="tot_sb")
        nc.vector.tensor_copy(out=tot_sb[:], in_=tot_ps[:1, :1])
        asum_ps = pp_pool.tile([P, MM_N], F32, tag="ps", name="asum_ps")
        nc.tensor.matmul(asum_ps[:, :1], ones_row[:], tot_sb[:])
        allsum = scal_pool.tile([P, 1], F32, tag="allsum")
        nc.vector.tensor_copy(out=allsum[:], in_=asum_ps[:, :1])
        # m = max(mean_abs, EPS); s_w = 1/m; deqm = m/127
        m_t = scal_pool.tile([P, 1], F32, tag="m_t")
        nc.vector.tensor_scalar(
            out=m_t[:], in0=allsum[:], scalar1=1.0 / (D * D), scalar2=EPS,
            op0=mybir.AluOpType.mult, op1=mybir.AluOpType.max,
        )
        s_w = _scale_chain(nc, scal_pool, m_t, "sw", 1.0)
        deqm = scal_pool.tile([P, 1], F32, tag="deqm")
        nc.vector.tensor_scalar_mul(deqm[:], m_t[:], 1.0 / 127.0)

        # reload streamed W tiles for ternarize; keep the Pool DMA queue
        # ordered [all 16 W loads] -> [reloads] so a reload's slot-wait
        # can't head-of-line-block the tail of the absmean load stream.
        prev_rel = wlast[0]
        for j in range(N_RES, n_wt):
            wf = wstr_pool.tile([P, D], F32, tag="wf32", name=f"wft_{j}")
            rl = pool_dma(wf[:], w_d[ts(j, P), :])
            add_dep_helper(rl.ins, prev_rel.ins, sync=False,
                           reason="reloads after W loads")
            prev_rel = rl
            wf_tiles[j] = wf

        # resident transposed ternary weights, classic chunk layout
        # [d0=128, dc, o]: d = 128*dc + d0.  Split per output-column block
        # so ob=0 matmuls unblock as soon as W tiles j=0..3 are ternarized
        # (tile-granular dependency tracking).
        wq_dt = FP8 if use_fp8 else BF16
        wqt_obs = [
            wq_pool.tile([P, n_dc, MM_N], wq_dt, tag="wqt", name=f"wqt_{ob}")
            for ob in range(n_ob)
        ]

        last_pass1 = [None]

        def ternarize(j):
            wf = wf_tiles[j]
            # pass1 (ACT, in-place f32): y' = relu(w*s_w + 1.49)
            # (= max(w*s_w, -1.49) + 1.49; the -1.49 re-shift is folded
            # into pass2's magic constant)
            last_pass1[0] = nc.scalar.activation(
                wf[:], wf[:], mybir.ActivationFunctionType.Relu,
                bias=clip_lo[:], scale=s_w[:],
            )
            for h in range(2):
                # pass2 (DVE): z = y' + (192 - 1.49) -> bf16 magic round
                wz_t = wz_pool.tile([P, HALF], BF16, tag="wz",
                                    name=f"wz_{j}_{h}")
                nc.vector.tensor_scalar_add(
                    wz_t[:], wf[:, ts(h, HALF)], MAGIC2
                )
                # transpose bf16, then pass3 (DVE): q = min(z - 192, 1)
                wzT = wzT_pool.tile([P, n_dc // 2, P], BF16, tag="wzT",
                                    name=f"wzT_{j}_{h}")
                nc.sync.dma_start_transpose(out=wzT[:], in_=wz_t[:])
                nc.vector.tensor_scalar(
                    out=wqt_obs[j // 4][:, 8 * h : 8 * h + 8,
                                        ts(j % 4, P)],
                    in0=wzT[:],
                    scalar1=-MAGIC16, scalar2=1.0,
                    op0=mybir.AluOpType.add, op1=mybir.AluOpType.min,
                )

        # ---------------- A_eff (only if nonzero A_raw) ----------------
        if with_h:
            a_d = io["a_raw"]
            h_d = io["h"]
            a1 = scal_pool.tile([1, D], F32, tag="a1")
            nc.sync.dma_start(out=a1[:], in_=a_d[:, :])
            aeff = scal_pool.tile([P, D], F32, tag="aeff")
            for ob in range(n_ob):
                ab_ps = pp_pool.tile([P, MM_N], F32, tag="ps",
                                     name=f"ab_ps{ob}")
                nc.tensor.matmul(ab_ps[:], ones_row[:], a1[:, ts(ob, MM_N)])
                nc.vector.tensor_copy(out=aeff[:, ts(ob, MM_N)], in_=ab_ps[:])
            nc.scalar.activation(
                aeff[:], aeff[:], mybir.ActivationFunctionType.Tanh
            )
            nc.vector.tensor_scalar_mul(aeff[:], aeff[:], 0.99)

        # ---------------- e quant pipeline ----------------
        eT_tiles = {}
        deq_tiles = {}
        rm_tiles = {}
        eq_ins = {}
        eq_dve = [None]
        scale_last = _SCALE_LAST
        act_gate = [None]  # ACT op that late equants must follow (last pass1)

        def equant(i, with_deq=True):
            # keep the e-load stream 3 blocks ahead (ef pool depth)
            if i + 3 < n_tb:
                ef3 = ef_pool.tile([P, D], F32, tag="ef", name=f"ef_{i+3}")
                nc.sync.dma_start(out=ef3[:], in_=e_d[ts(i + 3, P), :])
                ef_tiles[i + 3] = ef3
            ef = ef_tiles[i]
            rmax = st_pool.tile([P, 1], F32, tag="rmax")
            nc.vector.tensor_reduce(
                out=rmax[:], in_=ef[:], axis=mybir.AxisListType.X,
                op=mybir.AluOpType.max, apply_absolute_value=True,
            )
            rm_c = st_pool.tile([P, 1], F32, tag="rm_c")
            nc.vector.tensor_scalar_max(rm_c[:], rmax[:], EPS)
            rm_tiles[i] = rm_c
            scale = _scale_chain(nc, st_pool, rm_c, "sc", 127.0)
            eq_dve[0] = scale_last[0]
            if with_deq:
                eq_dve[0] = emit_deq(i)
            # magic round in f32 (ACT, in-place), -MAGIC -> bf16 (ACT)
            quantA = nc.scalar.activation(
                ef[:], ef[:], mybir.ActivationFunctionType.Identity,
                bias=posmagic[:], scale=scale[:],
            )
            qb = qb_pool.tile([P, D], BF16, tag="qb", name=f"qb_{i}")
            passB = nc.scalar.activation(
                qb[:], ef[:], mybir.ActivationFunctionType.Identity,
                bias=negmagic[:], scale=1.0,
            )
            eT = eT_pool.tile([P, n_dc, P], BF16, tag="eT", name=f"eT_{i}")
            for h in range(2):
                nc.sync.dma_start_transpose(
                    out=eT[:, 8 * h : 8 * h + 8, :], in_=qb[:, ts(h, HALF)]
                )
            cvt = passB
            if use_fp8:
                # small ints are exact; RNE convert bf16 -> fp8 on ACT,
                # in halves so the first matmul chunks can start as soon
                # as the low half is converted
                eT8 = eT8_pool.tile([P, n_dc, P], FP8, tag="eT8",
                                    name=f"eT8_{i}")
                for h in range(2):
                    cvt = nc.vector.tensor_copy(
                        out=eT8[:, 8 * h : 8 * h + 8, :],
                        in_=eT[:, 8 * h : 8 * h + 8, :],
                    )
                eT_tiles[i] = eT8
            else:
                eT_tiles[i] = eT
            # rotation hint: the previous block's cvt (which waits on its
            # transpose) must not head-of-line-block this block's quant
            if i - 1 in eq_ins:
                add_dep_helper(eq_ins[i - 1]["cvt"].ins, passB.ins,
                               sync=False,
                               reason="cvt(i-1) after passB(i) on ACT")
            eq_ins[i] = {"passB": passB, "cvt": cvt, "dve": eq_dve[0]}

        def emit_deq(i):
            deq = st_pool.tile([P, 1], F32, tag="deq")
            op = nc.vector.tensor_scalar_mul(deq[:], rm_tiles[i][:], deqm[:])
            deq_tiles[i] = deq
            return op

        # blocks 0..2 quantize entirely during the W-load phase: ACT and
        # SP are idle there, and nothing in the quant chain except deq
        # needs the global W scale.  (Their quantA emissions precede the
        # act_gate assignment, so they are NOT gated behind W pass1.)
        n_pre = min(3, n_tb)
        for i in range(n_pre):
            equant(i, with_deq=False)
        for i in range(n_pre):
            emit_deq(i)

        # ---------------- W ternarize ----------------
        for j in range(n_wt):
            ternarize(j)

        # ---------------- main token-block loop ----------------
        for i in range(n_tb):
            if n_pre <= i + 2 < n_tb:
                equant(i + 2)
            if i + 1 < n_tb:
                bo_t = bo_pool.tile([P, D], F32, tag="bo", name=f"bo_{i+1}")
                pool_dma(bo_t[:], bo_d[ts(i + 1, P), :])
                bo_tiles[i + 1] = bo_t

            bo_t = bo_tiles[i]
            deq = deq_tiles[i]
            for ob in range(n_ob):
                ps = pp_pool.tile([P, MM_N], F32, tag="ps", name=f"ps{i}_{ob}")
                if use_fp8:
                    eT8 = eT_tiles[i]
                    for c in range(n_pc):
                        nc.tensor.matmul(
                            ps[:],
                            eT8[:, 2 * c : 2 * c + 2, :],
                            wqt_obs[ob][:, 2 * c : 2 * c + 2, :],
                            start=(c == 0), stop=(c == n_pc - 1),
                            perf_mode=mybir.MatmulPerfMode.DoubleRow,
                        )
                else:
                    eT = eT_tiles[i]
                    for d in range(n_dc):
                        nc.tensor.matmul(
                            ps[:], eT[:, d, :], wqt_obs[ob][:, d, :],
                            start=(d == 0), stop=(d == n_dc - 1),
                        )
                # bo = psum * deq + block_out (fused dequant + add)
                epi = nc.vector.scalar_tensor_tensor(
                    out=bo_t[:, ts(ob, MM_N)], in0=ps[:], scalar=deq[:],
                    in1=bo_t[:, ts(ob, MM_N)],
                    op0=mybir.AluOpType.mult, op1=mybir.AluOpType.add,
                )
                if i + 1 in eq_ins:
                    add_dep_helper(epi.ins, eq_ins[i + 1]["dve"].ins,
                                   sync=False,
                                   reason="epi(i) after quant dve(i+1)")
            nc.sync.dma_start(out=out_d[ts(i, P), :], in_=bo_t[:])


def legalize_waits(nc):
    """Walrus in this container encodes at most ONE sync wait per ISA
    instruction (the 64B Events field) and refuses to split.  Rewrite any
    instruction carrying N>1 waits into N-1 single-wait NOP carrier
    instructions on the same engine placed immediately before it, keeping one
    wait on the original.  Waits are monotonic sem>=v conditions, so splitting
    preserves semantics exactly."""
    import bass_rust

    eng_map = {
        mybir.EngineType.SP: nc.sync,
        mybir.EngineType.DVE: nc.vector,
        mybir.EngineType.Activation: nc.scalar,
        mybir.EngineType.PE: nc.tensor,
        mybir.EngineType.Pool: nc.gpsimd,
    }
    for f in nc.m.functions:
        for blk in f.blocks:
            insts = list(blk.instructions)
            if not any(
                i.sync_info is not None and len(i.sync_info.on_wait) > 1
                for i in insts
            ):
                continue
            carriers = {}  # target inst name -> list of carrier insts
            for inst in insts:
                si = inst.sync_info
                if si is None or len(si.on_wait) <= 1:
                    continue
                waits = list(si.on_wait)
                cs = []
                for w in waits[:-1]:
                    bi = eng_map[inst.engine].nop(nofuse=True)
                    nop_inst = bi.ins
                    nop_inst.sync_info = bass_rust.SyncInfo(
                        on_wait=[w], on_update=[]
                    )
                    cs.append(nop_inst)
                carriers[inst.name] = cs
                inst.sync_info = bass_rust.SyncInfo(
                    on_wait=[waits[-1]], on_update=list(si.on_update)
                )
            # nops were appended to the current bb; remove them from wherever
            # they landed and splice before their targets.
            carrier_names = {c.name for cs in carriers.values() for c in cs}
            for f2 in nc.m.functions:
                for blk2 in f2.blocks:
                    cur = list(blk2.instructions)
                    if any(i.name in carrier_names for i in cur):
                        blk2.instructions = [
                            i for i in cur if i.name not in carrier_names
                        ]
            new_list = []
            for inst in blk.instructions:
                for c in carriers.get(inst.name, ()):
                    new_list.append(c)
                new_list.append(inst)
            blk.instructions = new_list


def build_nc(Tc: int, D: int, with_h: bool, use_fp8: bool):
    nc = bass.Bass("TRN2", target_bir_lowering=False, debug=False)
    io = {
        "e": nc.declare_dram_parameter("e", [Tc, D], F32, isOutput=False)[:],
        "bo": nc.declare_dram_parameter("bo", [Tc, D], F32, isOutput=False)[:],
        "w": nc.declare_dram_parameter("w", [D, D], F32, isOutput=False)[:],
    }
    if with_h:
        io["h"] = nc.declare_dram_parameter("h", [Tc, D], F32, isOutput=False)[:]
        io["a_raw"] = nc.declare_dram_parameter(
            "a_raw", [1, D], F32, isOutput=False
        )[:]
    io["out"] = nc.declare_dram_parameter("out", [Tc, D], F32, isOutput=True)[:]
    with tile.TileContext(nc) as tc:
        build_kernel_body(tc, io, Tc, D, with_h, use_fp8)
    legalize_waits(nc)
    return nc


_NC_CACHE: dict = {}


def _get_nc(Tc: int, D: int, with_h: bool, use_fp8: bool):
    key = (Tc, D, with_h, use_fp8)
    if key not in _NC_CACHE:
        _NC_CACHE[key] = build_nc(Tc, D, with_h, use_fp8)
    return _NC_CACHE[key]


def kernel(h, e, block_out, A_raw, W, _trace=False, _trace_kwargs=None):
    Bb, Tt, D = e.shape
    rows = Bb * Tt
    Tc = rows // N_CORES
    e2 = e.reshape(rows, D)
    bo2 = block_out.reshape(rows, D)
    h2 = h.reshape(rows, D)
    use_fp8 = os.environ.get("KERNEL_VARIANT", "fp8") != "bf16"

    # A_raw is zero-initialized in this module, so A_eff*h vanishes; if a
    # caller ever passes a nonzero A_raw, fold the (cheap, elementwise)
    # A_eff*h term into block_out on the host and run the same kernel.
    if np.any(A_raw):
        aeff = (0.99 * np.tanh(A_raw.astype(np.float32))).astype(np.float32)
        bo2 = bo2 + aeff[None, :] * h2

    nc = _get_nc(Tc, D, False, use_fp8)
    in_maps = []
    for c in range(N_CORES):
        sl = slice(c * Tc, (c + 1) * Tc)
        m = {
            "e": np.ascontiguousarray(e2[sl]),
            "bo": np.ascontiguousarray(bo2[sl]),
            "w": np.ascontiguousarray(W),
        }
        in_maps.append(m)

    res = run_bass_kernel_spmd(
        nc, in_maps, list(range(N_CORES)), trace=_trace,
        **(_trace_kwargs or {}),
    )
    out = np.concatenate(
        [res.results[c]["out"] for c in range(N_CORES)], axis=0
    )
    if _trace:
        return out.reshape(Bb, Tt, D), res
    return out.reshape(Bb, Tt, D)


# revision 62
# speedup vs baseline: 1.0140x; 1.0140x over previous
"""Trainium2 Bass kernel for BitLTIInjection (BitNet-style fake-quantized linear
+ LTI injection):

    A_eff = 0.99*tanh(A_raw)
    e_q   = per-token absmax int8 fake quant of e
    W_q   = absmean ternary fake quant of W
    out   = A_eff*h + e_q @ W_q.T + block_out

Data-parallel over B*T across 8 cores; W replicated.

Primary variant ("fp8"): the quantized matmul runs in fp8e4m3 with the
DoubleRow (double-pumped) PE mode, contracting two 128-chunks per pass.
W_q is ternary {-1,0,1} (exact in fp8); e_q loses ~1.5e-2 relative error
from fp8 rounding of int8 values, well inside the 2e-2 gate.  Operands
use the classic chunk layout [128, n_chunks, free] (d = 128*dc + p) with
the pair dim sliced [2c:2c+2] — the hardware's dual-fp8 LDWEIGHTS ISA
check rejects pair-interleaved layouts, so transposes run in bf16 and a
cheap convert pass produces the fp8 copies.

Fallback variant ("bf16"): numerically exact bf16 matmul (all quantized
values are small ints, bf16 + f32 PSUM is exact), same scheduling.

Scheduling: W loads stream on the Pool DMA queue with the absmean
reduce overlapping on DVE; the first three e blocks are loaded and
quantized during that window (only the dequant scalar needs the global
W scale, so it is deferred).  Ternarize is a 3-pass pipeline: pass1
relu(w*s + 1.49) on ACT (the low-clip reshift folds into the magic
constant), pass2 +190.51 bf16 magic-round on DVE, XBAR transpose on SP,
pass3 -192/min(1) -> fp8 on DVE.  wqt is split per output-column block
so ob=0 matmuls unblock after four W tiles.  e loads ride the SP queue,
out-stores the ACT queue, bo/W the Pool queue; no-sync dep hints keep
reloads behind the W load stream and keep each block's tail ops (cvt,
epilogues) from head-of-line-blocking the next block's quant chain.
Most W f32 tiles stay SBUF-resident between absmean and ternarize.
"""

import os

import numpy as np

import concourse.bass as bass
import concourse.mybir as mybir
import concourse.tile as tile
from concourse.bass import ts
from concourse.bass_utils import run_bass_kernel_spmd
from concourse.tile_rust import add_dep_helper

P = 128
MAGIC = 12582912.0   # 1.5 * 2**23: forces RNE-to-integer in f32
MAGIC16 = 192.0      # 1.5 * 2**7: forces RNE-to-integer in bf16 for |x| < 64
EPS = 1e-5
N_CORES = 8
F32 = mybir.dt.float32
BF16 = mybir.dt.bfloat16
FP8 = mybir.dt.float8e4
U16 = mybir.dt.uint16
MM_N = 512  # moving free dim per matmul (one PSUM bank of f32)
CLIP_LO = 1.49  # pre-round low clip offset (relu(w*s + 1.49))
MAGIC2 = float(np.float32(MAGIC16) - np.float32(CLIP_LO))


_SCALE_LAST = [None]


def _scale_chain(nc, pool, m_in, tag, mul):
    """1/m_in via reciprocal + one Newton step, times mul.  Returns AP."""
    r0 = pool.tile([P, 1], F32, tag=f"{tag}_r0")
    nc.vector.reciprocal(r0[:], m_in[:])
    t1 = pool.tile([P, 1], F32, tag=f"{tag}_t1")
    nc.vector.scalar_tensor_tensor(
        out=t1[:], in0=m_in[:], scalar=-1.0, in1=r0[:],
        op0=mybir.AluOpType.mult, op1=mybir.AluOpType.mult,
    )
    nc.vector.tensor_scalar_add(t1[:], t1[:], 2.0)
    op = nc.vector.tensor_scalar_mul(r0[:], r0[:], t1[:])
    if mul != 1.0:
        op = nc.vector.tensor_scalar_mul(r0[:], r0[:], mul)
    _SCALE_LAST[0] = op
    return r0


def build_kernel_body(tc: tile.TileContext, io: dict, Tc: int, D: int,
                      with_h: bool, use_fp8: bool):
    nc = tc.nc
    n_tb = Tc // P   # token blocks per core
    n_wt = D // P    # weight row tiles
    n_ob = D // MM_N  # output column blocks
    n_pc = D // 256  # fp8 pair-chunks
    n_dc = D // P    # bf16 contraction chunks
    HALF = D // 2

    # W f32 tiles resident in SBUF between absmean and ternarize; the rest
    # stream through 2 slots and are re-read from HBM for ternarize.
    N_RES = 9 if use_fp8 else 6

    e_d = io["e"]
    bo_d = io["bo"]
    w_d = io["w"]
    out_d = io["out"]

    with (
        tc.tile_pool(name="wres", bufs=N_RES) as wres_pool,
        tc.tile_pool(name="wstr", bufs=2) as wstr_pool,
        tc.tile_pool(name="wq", bufs=4) as wq_pool,
        tc.tile_pool(name="wz", bufs=2) as wz_pool,
        tc.tile_pool(name="wzT", bufs=2) as wzT_pool,
        tc.tile_pool(name="eT8", bufs=6) as eT8_pool,
        tc.tile_pool(name="scal", bufs=1) as scal_pool,
        tc.tile_pool(name="st", bufs=4) as st_pool,
        tc.tile_pool(name="ef", bufs=3) as ef_pool,
        tc.tile_pool(name="qb", bufs=3) as qb_pool,
        tc.tile_pool(name="eT", bufs=3) as eT_pool,
        tc.tile_pool(name="bo", bufs=2) as bo_pool,
        tc.tile_pool(name="pp", bufs=8, space="PSUM") as pp_pool,
    ):
        # constants
        ones_col = scal_pool.tile([P, 1], F32, tag="ones_col")
        nc.vector.memset(ones_col[:], 1.0)
        ones_row = scal_pool.tile([1, P], F32, tag="ones_row")
        nc.vector.memset(ones_row[:], 1.0)
        clip_lo = scal_pool.tile([P, 1], F32, tag="clip_lo")
        nc.vector.memset(clip_lo[:], CLIP_LO)
        posmagic = scal_pool.tile([P, 1], F32, tag="posmagic")
        nc.vector.memset(posmagic[:], MAGIC)
        negmagic = scal_pool.tile([P, 1], F32, tag="negmagic")
        nc.vector.memset(negmagic[:], -MAGIC)

        # ---------------- W load + absmean (overlapped) ----------------
        # The first N_RES tiles stay SBUF-resident; the last few stream
        # through 2 slots and are re-read from HBM for ternarize.  All
        # reduces on DVE (no cross-engine wait can wedge the pipeline).
        parts = scal_pool.tile([P, n_wt], F32, tag="parts")
        wf_tiles = {}
        # ALL HBM DMA goes through the Pool queue: a single queue sustains
        # ~357 GB/s while splitting traffic across queues DROPS aggregate
        # throughput (measured).  The FIFO order is pinned with no-sync
        # dep hints so a slot-blocked dispatch can't reorder ahead.
        def pool_dma(out_ap, in_ap):
            return nc.gpsimd.dma_start(out=out_ap, in_=in_ap)

        ef_tiles = {}
        wlast = [None]
        for j in range(n_wt):
            pool = wres_pool if j < N_RES else wstr_pool
            wf = pool.tile([P, D], F32, tag="wf32", name=f"wfm_{j}")
            wlast[0] = pool_dma(wf[:], w_d[ts(j, P), :])
            nc.vector.tensor_reduce(
                out=parts[:, j : j + 1], in_=wf[:],
                axis=mybir.AxisListType.X, op=mybir.AluOpType.add,
                apply_absolute_value=True,
            )
            if j < N_RES:
                wf_tiles[j] = wf
            # interleave the first three e loads so the pre-quant chain
            # (blocks 0-2) can run during the W-load window
            if j in (3, 7, 11) and (j - 3) // 4 < n_tb:
                i = (j - 3) // 4
                ef = ef_pool.tile([P, D], F32, tag="ef", name=f"ef_{i}")
                nc.sync.dma_start(out=ef[:], in_=e_d[ts(i, P), :])
                ef_tiles[i] = ef

        bo_tiles = {}
        bo0 = bo_pool.tile([P, D], F32, tag="bo", name="bo_0")
        pool_dma(bo0[:], bo_d[ts(0, P), :])
        bo_tiles[0] = bo0

        # absmean finalize: cross-partition sum + broadcast via tiny PE ops
        acc = scal_pool.tile([P, 1], F32, tag="acc")
        nc.vector.tensor_reduce(
            out=acc[:], in_=parts[:], axis=mybir.AxisListType.X,
            op=mybir.AluOpType.add,
        )
        tot_ps = pp_pool.tile([P, MM_N], F32, tag="ps", name="tot_ps")
        nc.tensor.matmul(tot_ps[:1, :1], ones_col[:], acc[:])
        tot_sb = scal_pool.tile([1, 1], F32, tag="tot_sb")
        nc.vector.tensor_copy(out=tot_sb[:], in_=tot_ps[:1, :1])
        asum_ps = pp_pool.tile([P, MM_N], F32, tag="ps", name="asum_ps")
        nc.tensor.matmul(asum_ps[:, :1], ones_row[:], tot_sb[:])
        allsum = scal_pool.tile([P, 1], F32, tag="allsum")
        nc.vector.tensor_copy(out=allsum[:], in_=asum_ps[:, :1])
        # m = max(mean_abs, EPS); s_w = 1/m; deqm = m/127
        m_t = scal_pool.tile([P, 1], F32, tag="m_t")
        nc.vector.tensor_scalar(
            out=m_t[:], in0=allsum[:], scalar1=1.0 / (D * D), scalar2=EPS,
            op0=mybir.AluOpType.mult, op1=mybir.AluOpType.max,
        )
        s_w = _scale_chain(nc, scal_pool, m_t, "sw", 1.0)
        deqm = scal_pool.tile([P, 1], F32, tag="deqm")
        nc.vector.tensor_scalar_mul(deqm[:], m_t[:], 1.0 / 127.0)

        # reload streamed W tiles for ternarize; keep the Pool DMA queue
        # ordered [all 16 W loads] -> [reloads] so a reload's slot-wait
        # can't head-of-line-block the tail of the absmean load stream.
        prev_rel = wlast[0]
        for j in range(N_RES, n_wt):
            wf = wstr_pool.tile([P, D], F32, tag="wf32", name=f"wft_{j}")
            rl = pool_dma(wf[:], w_d[ts(j, P), :])
            add_dep_helper(rl.ins, prev_rel.ins, sync=False,
                           reason="reloads after W loads")
            prev_rel = rl
            wf_tiles[j] = wf

        # resident transposed ternary weights, classic chunk layout
        # [d0=128, dc, o]: d = 128*dc + d0.  Split per output-column block
        # so ob=0 matmuls unblock as soon as W tiles j=0..3 are ternarized
        # (tile-granular dependency tracking).
        wq_dt = FP8 if use_fp8 else BF16
        wqt_obs = [
            wq_pool.tile([P, n_dc, MM_N], wq_dt, tag="wqt", name=f"wqt_{ob}")
            for ob in range(n_ob)
        ]

        last_pass1 = [None]

        def ternarize(j):
            wf = wf_tiles[j]
            # pass1 (ACT, in-place f32): y' = relu(w*s_w + 1.49)
            # (= max(w*s_w, -1.49) + 1.49; the -1.49 re-shift is folded
            # into pass2's magic constant)
            last_pass1[0] = nc.scalar.activation(
                wf[:], wf[:], mybir.ActivationFunctionType.Relu,
                bias=clip_lo[:], scale=s_w[:],
            )
            for h in range(2):
                # pass2 (DVE): z = y' + (192 - 1.49) -> bf16 magic round
                wz_t = wz_pool.tile([P, HALF], BF16, tag="wz",
                                    name=f"wz_{j}_{h}")
                nc.vector.tensor_scalar_add(
                    wz_t[:], wf[:, ts(h, HALF)], MAGIC2
                )
                # transpose bf16, then pass3 (DVE): q = min(z - 192, 1)
                wzT = wzT_pool.tile([P, n_dc // 2, P], BF16, tag="wzT",
                                    name=f"wzT_{j}_{h}")
                nc.sync.dma_start_transpose(out=wzT[:], in_=wz_t[:])
                nc.vector.tensor_scalar(
                    out=wqt_obs[j // 4][:, 8 * h : 8 * h + 8,
                                        ts(j % 4, P)],
                    in0=wzT[:],
                    scalar1=-MAGIC16, scalar2=1.0,
                    op0=mybir.AluOpType.add, op1=mybir.AluOpType.min,
                )

        # ---------------- A_eff (only if nonzero A_raw) ----------------
        if with_h:
            a_d = io["a_raw"]
            h_d = io["h"]
            a1 = scal_pool.tile([1, D], F32, tag="a1")
            nc.sync.dma_start(out=a1[:], in_=a_d[:, :])
            aeff = scal_pool.tile([P, D], F32, tag="aeff")
            for ob in range(n_ob):
                ab_ps = pp_pool.tile([P, MM_N], F32, tag="ps",
                                     name=f"ab_ps{ob}")
                nc.tensor.matmul(ab_ps[:], ones_row[:], a1[:, ts(ob, MM_N)])
                nc.vector.tensor_copy(out=aeff[:, ts(ob, MM_N)], in_=ab_ps[:])
            nc.scalar.activation(
                aeff[:], aeff[:], mybir.ActivationFunctionType.Tanh
            )
            nc.vector.tensor_scalar_mul(aeff[:], aeff[:], 0.99)

        # ---------------- e quant pipeline ----------------
        eT_tiles = {}
        deq_tiles = {}
        rm_tiles = {}
        eq_ins = {}
        eq_dve = [None]
        scale_last = _SCALE_LAST
        act_gate = [None]  # ACT op that late equants must follow (last pass1)

        def equant(i, with_deq=True):
            # keep the e-load stream 3 blocks ahead (ef pool depth)
            if i + 3 < n_tb:
                ef3 = ef_pool.tile([P, D], F32, tag="ef", name=f"ef_{i+3}")
                nc.sync.dma_start(out=ef3[:], in_=e_d[ts(i + 3, P), :])
                ef_tiles[i + 3] = ef3
            ef = ef_tiles[i]
            rmax = st_pool.tile([P, 1], F32, tag="rmax")
            nc.vector.tensor_reduce(
                out=rmax[:], in_=ef[:], axis=mybir.AxisListType.X,
                op=mybir.AluOpType.max, apply_absolute_value=True,
            )
            rm_c = st_pool.tile([P, 1], F32, tag="rm_c")
            nc.vector.tensor_scalar_max(rm_c[:], rmax[:], EPS)
            rm_tiles[i] = rm_c
            scale = _scale_chain(nc, st_pool, rm_c, "sc", 127.0)
            eq_dve[0] = scale_last[0]
            if with_deq:
                eq_dve[0] = emit_deq(i)
            # magic round in f32 (ACT, in-place), -MAGIC -> bf16 (ACT)
            quantA = nc.scalar.activation(
                ef[:], ef[:], mybir.ActivationFunctionType.Identity,
                bias=posmagic[:], scale=scale[:],
            )
            qb = qb_pool.tile([P, D], BF16, tag="qb", name=f"qb_{i}")
            passB = nc.scalar.activation(
                qb[:], ef[:], mybir.ActivationFunctionType.Identity,
                bias=negmagic[:], scale=1.0,
            )
            eT = eT_pool.tile([P, n_dc, P], BF16, tag="eT", name=f"eT_{i}")
            nc.sync.dma_start_transpose(out=eT[:], in_=qb[:])
            cvt = passB
            if use_fp8:
                # small ints are exact; RNE convert bf16 -> fp8 on ACT
                eT8 = eT8_pool.tile([P, n_dc, P], FP8, tag="eT8",
                                    name=f"eT8_{i}")
                cvt = nc.scalar.activation(
                    eT8[:], eT[:], mybir.ActivationFunctionType.Identity,
                )
                eT_tiles[i] = eT8
            else:
                eT_tiles[i] = eT
            # rotation hint: the previous block's cvt (which waits on its
            # transpose) must not head-of-line-block this block's quant
            if i - 1 in eq_ins:
                add_dep_helper(eq_ins[i - 1]["cvt"].ins, passB.ins,
                               sync=False,
                               reason="cvt(i-1) after passB(i) on ACT")
            eq_ins[i] = {"passB": passB, "cvt": cvt, "dve": eq_dve[0]}

        def emit_deq(i):
            deq = st_pool.tile([P, 1], F32, tag="deq")
            op = nc.vector.tensor_scalar_mul(deq[:], rm_tiles[i][:], deqm[:])
            deq_tiles[i] = deq
            return op

        # blocks 0..2 quantize entirely during the W-load phase: ACT and
        # SP are idle there, and nothing in the quant chain except deq
        # needs the global W scale.  (Their quantA emissions precede the
        # act_gate assignment, so they are NOT gated behind W pass1.)
        n_pre = min(3, n_tb)
        for i in range(n_pre):
            equant(i, with_deq=False)
        for i in range(n_pre):
            emit_deq(i)

        # ---------------- W ternarize ----------------
        for j in range(n_wt):
            ternarize(j)

        # ---------------- main token-block loop ----------------
        for i in range(n_tb):
            if n_pre <= i + 2 < n_tb:
                equant(i + 2)
            if i + 1 < n_tb:
                bo_t = bo_pool.tile([P, D], F32, tag="bo", name=f"bo_{i+1}")
                pool_dma(bo_t[:], bo_d[ts(i + 1, P), :])
                bo_tiles[i + 1] = bo_t

            bo_t = bo_tiles[i]
            deq = deq_tiles[i]
            for ob in range(n_ob):
                ps = pp_pool.tile([P, MM_N], F32, tag="ps", name=f"ps{i}_{ob}")
                if use_fp8:
                    eT8 = eT_tiles[i]
                    for c in range(n_pc):
                        nc.tensor.matmul(
                            ps[:],
                            eT8[:, 2 * c : 2 * c + 2, :],
                            wqt_obs[ob][:, 2 * c : 2 * c + 2, :],
                            start=(c == 0), stop=(c == n_pc - 1),
                            perf_mode=mybir.MatmulPerfMode.DoubleRow,
                        )
                else:
                    eT = eT_tiles[i]
                    for d in range(n_dc):
                        nc.tensor.matmul(
                            ps[:], eT[:, d, :], wqt_obs[ob][:, d, :],
                            start=(d == 0), stop=(d == n_dc - 1),
                        )
                # bo = psum * deq + block_out (fused dequant + add)
                epi = nc.vector.scalar_tensor_tensor(
                    out=bo_t[:, ts(ob, MM_N)], in0=ps[:], scalar=deq[:],
                    in1=bo_t[:, ts(ob, MM_N)],
                    op0=mybir.AluOpType.mult, op1=mybir.AluOpType.add,
                )
                if i + 1 in eq_ins:
                    add_dep_helper(epi.ins, eq_ins[i + 1]["dve"].ins,
                                   sync=False,
                                   reason="epi(i) after quant dve(i+1)")
            nc.scalar.dma_start(out=out_d[ts(i, P), :], in_=bo_t[:])


def legalize_waits(nc):
    """Walrus in this container encodes at most ONE sync wait per ISA
    instruction (the 64B Events field) and refuses to split.  Rewrite any
    instruction carrying N>1 waits into N-1 single-wait NOP carrier
    instructions on the same engine placed immediately before it, keeping one
    wait on the original.  Waits are monotonic sem>=v conditions, so splitting
    preserves semantics exactly."""
    import bass_rust

    eng_map = {
        mybir.EngineType.SP: nc.sync,
        mybir.EngineType.DVE: nc.vector,
        mybir.EngineType.Activation: nc.scalar,
        mybir.EngineType.PE: nc.tensor,
        mybir.EngineType.Pool: nc.gpsimd,
    }
    for f in nc.m.functions:
        for blk in f.blocks:
            insts = list(blk.instructions)
            if not any(
                i.sync_info is not None and len(i.sync_info.on_wait) > 1
                for i in insts
            ):
                continue
            carriers = {}  # target inst name -> list of carrier insts
            for inst in insts:
                si = inst.sync_info
                if si is None or len(si.on_wait) <= 1:
                    continue
                waits = list(si.on_wait)
                cs = []
                for w in waits[:-1]:
                    bi = eng_map[inst.engine].nop(nofuse=True)
                    nop_inst = bi.ins
                    nop_inst.sync_info = bass_rust.SyncInfo(
                        on_wait=[w], on_update=[]
                    )
                    cs.append(nop_inst)
                carriers[inst.name] = cs
                inst.sync_info = bass_rust.SyncInfo(
                    on_wait=[waits[-1]], on_update=list(si.on_update)
                )
            # nops were appended to the current bb; remove them from wherever
            # they landed and splice before their targets.
            carrier_names = {c.name for cs in carriers.values() for c in cs}
            for f2 in nc.m.functions:
                for blk2 in f2.blocks:
                    cur = list(blk2.instructions)
                    if any(i.name in carrier_names for i in cur):
                        blk2.instructions = [
                            i for i in cur if i.name not in carrier_names
                        ]
            new_list = []
            for inst in blk.instructions:
                for c in carriers.get(inst.name, ()):
                    new_list.append(c)
                new_list.append(inst)
            blk.instructions = new_list


def build_nc(Tc: int, D: int, with_h: bool, use_fp8: bool):
    nc = bass.Bass("TRN2", target_bir_lowering=False, debug=False)
    io = {
        "e": nc.declare_dram_parameter("e", [Tc, D], F32, isOutput=False)[:],
        "bo": nc.declare_dram_parameter("bo", [Tc, D], F32, isOutput=False)[:],
        "w": nc.declare_dram_parameter("w", [D, D], F32, isOutput=False)[:],
    }
    if with_h:
        io["h"] = nc.declare_dram_parameter("h", [Tc, D], F32, isOutput=False)[:]
        io["a_raw"] = nc.declare_dram_parameter(
            "a_raw", [1, D], F32, isOutput=False
        )[:]
    io["out"] = nc.declare_dram_parameter("out", [Tc, D], F32, isOutput=True)[:]
    with tile.TileContext(nc) as tc:
        build_kernel_body(tc, io, Tc, D, with_h, use_fp8)
    legalize_waits(nc)
    return nc


_NC_CACHE: dict = {}


def _get_nc(Tc: int, D: int, with_h: bool, use_fp8: bool):
    key = (Tc, D, with_h, use_fp8)
    if key not in _NC_CACHE:
        _NC_CACHE[key] = build_nc(Tc, D, with_h, use_fp8)
    return _NC_CACHE[key]


def kernel(h, e, block_out, A_raw, W, _trace=False, _trace_kwargs=None):
    Bb, Tt, D = e.shape
    rows = Bb * Tt
    Tc = rows // N_CORES
    e2 = e.reshape(rows, D)
    bo2 = block_out.reshape(rows, D)
    h2 = h.reshape(rows, D)
    use_fp8 = os.environ.get("KERNEL_VARIANT", "fp8") != "bf16"

    # A_raw is zero-initialized in this module, so A_eff*h vanishes; if a
    # caller ever passes a nonzero A_raw, fold the (cheap, elementwise)
    # A_eff*h term into block_out on the host and run the same kernel.
    if np.any(A_raw):
        aeff = (0.99 * np.tanh(A_raw.astype(np.float32))).astype(np.float32)
        bo2 = bo2 + aeff[None, :] * h2

    nc = _get_nc(Tc, D, False, use_fp8)
    in_maps = []
    for c in range(N_CORES):
        sl = slice(c * Tc, (c + 1) * Tc)
        m = {
            "e": np.ascontiguousarray(e2[sl]),
            "bo": np.ascontiguousarray(bo2[sl]),
            "w": np.ascontiguousarray(W),
        }
        in_maps.append(m)

    res = run_bass_kernel_spmd(
        nc, in_maps, list(range(N_CORES)), trace=_trace,
        **(_trace_kwargs or {}),
    )
    out = np.concatenate(
        [res.results[c]["out"] for c in range(N_CORES)], axis=0
    )
    if _trace:
        return out.reshape(Bb, Tt, D), res
    return out.reshape(Bb, Tt, D)
